# revision 2
# baseline (speedup 1.0000x reference)
"""Trainium2 Bass kernel for the ButterflyModule problem.

Semantics (N=4096 rows, B=8192 cols):
  x = data[indices_in]
  4 Givens-rotation butterfly layers (strides 1,2,4,8 within 16-row blocks)
  bias + smooth-ReLU on rows with (row%16)<8
  4 more butterfly layers (strides 1,2,4,8)
  out = data with rows idx_out replaced by the result

Device strategy: the 4 input layers compose into a dense 16x16 matrix per
16-row block (256 blocks), same for the 4 output layers.  Each 128-row group
is then one block-diagonal 128x128 matmul on the TensorEngine.  The
activation folds into per-partition scalars:

  y' = D.Min @ x + D.b          (D = diag(0.5 on act rows, 1 elsewhere))
  u  = m * y'                   (m = 1 on act rows, 0 elsewhere; ACT Square scale)
  s  = sqrt(u^2 + (0.05)^2 * m) (ACT Sqrt with per-partition bias)
  z  = y' + s                   (act rows: 0.5*(xa+sqrt(xa^2+0.01)); else y)
  out = Mout @ z

Rows are sharded across the 8 cores (512 rows each); rotations never cross
16-row block boundaries so there is no cross-core communication.
"""

import sys

if "/opt/trn_rl_repo" not in sys.path:
    sys.path.insert(0, "/opt/trn_rl_repo")

import numpy as np

N_ROWS = 4096
N_COLS = 8192
COL_BLOCK = 16
NUM_ACT = 8
CURVATURE = 0.1
N_CORES = 8
ROWS_PER_CORE = N_ROWS // N_CORES          # 512
GROUPS_PER_CORE = ROWS_PER_CORE // 128     # 4
FREE = 512                                 # matmul moving-dim tile (fp32 max)
N_FTILES = N_COLS // FREE                  # 16

_PROGRAM_CACHE = {}


def _butterfly_mats(angles64):
    """Compose butterfly layers into per-block 16x16 matrices.

    angles64: [8, 2048] float64.  Returns (Min, Mout) each [256, 16, 16],
    where layer l uses stride 1<<(l%4) and block b uses angles[l, 8b:8b+8]
    ordered by the low row index within the block.
    """
    nb = N_ROWS // COL_BLOCK

    def accum(l0, l1):
        G = np.broadcast_to(np.eye(COL_BLOCK), (nb, COL_BLOCK, COL_BLOCK)).copy()
        for l in range(l0, l1):
            stride = 1 << (l % 4)
            offs = [o for o in range(COL_BLOCK) if (o & stride) == 0]
            a = angles64[l].reshape(nb, NUM_ACT)
            c = np.cos(a)
            s = np.sin(a)
            for k, o in enumerate(offs):
                gl = G[:, o, :].copy()
                gh = G[:, o + stride, :].copy()
                G[:, o, :] = c[:, k, None] * gl + s[:, k, None] * gh
                G[:, o + stride, :] = -s[:, k, None] * gl + c[:, k, None] * gh
        return G

    return accum(0, 4), accum(4, 8)


def _host_weights(angles, biases):
    """Build per-core weight tensors for the device kernel."""
    ang64 = np.asarray(angles, np.float64)
    b64 = np.asarray(biases, np.float64)
    Min, Mout = _butterfly_mats(ang64)

    off = np.arange(COL_BLOCK)
    d16 = np.where(off < NUM_ACT, 0.5, 1.0)          # post-layer-4 scale
    Minp = Min * d16[None, :, None]                  # diag(d) @ Min (row scale)

    MinT = Minp.transpose(0, 2, 1)                   # per-block lhsT
    MoutT = Mout.transpose(0, 2, 1)

    def block_diag8(blocksT):
        out = np.zeros((128, 128))
        for i in range(8):
            out[i * 16:(i + 1) * 16, i * 16:(i + 1) * 16] = blocksT[i]
        return out

    # bias' = d * bias on act rows, 0 elsewhere, laid out over all 4096 rows
    bf = np.zeros(N_ROWS)
    blk = np.arange(N_ROWS // COL_BLOCK)
    for o in range(NUM_ACT):
        bf[blk * COL_BLOCK + o] = 0.5 * b64[blk * NUM_ACT + o]

    p = np.arange(128)
    act = (p % COL_BLOCK) < NUM_ACT
    m_vec = np.where(act, 1.0, 0.0).astype(np.float32).reshape(128, 1)
    sqb_vec = np.where(act, (0.5 * CURVATURE) ** 2, 0.0).astype(np.float32)
    sqb_vec = sqb_vec.reshape(128, 1)

    per_core = []
    for c in range(N_CORES):
        win = np.zeros((GROUPS_PER_CORE, 128, 128))
        wout = np.zeros((GROUPS_PER_CORE, 128, 128))
        for g in range(GROUPS_PER_CORE):
            g_glob = c * GROUPS_PER_CORE + g
            win[g] = block_diag8(MinT[g_glob * 8:(g_glob + 1) * 8])
            wout[g] = block_diag8(MoutT[g_glob * 8:(g_glob + 1) * 8])
        win_dram = win.transpose(1, 0, 2).reshape(128, GROUPS_PER_CORE * 128)
        wout_dram = wout.transpose(1, 0, 2).reshape(128, GROUPS_PER_CORE * 128)
        bias_dram = (
            bf[c * ROWS_PER_CORE:(c + 1) * ROWS_PER_CORE]
            .reshape(GROUPS_PER_CORE, 128)
            .T
        )
        per_core.append(
            {
                "win": np.ascontiguousarray(win_dram, dtype=np.float32),
                "wout": np.ascontiguousarray(wout_dram, dtype=np.float32),
                "biasv": np.ascontiguousarray(bias_dram, dtype=np.float32),
                "mvec": m_vec,
                "sqbv": sqb_vec,
            }
        )
    return per_core


def _build_program(reps=None, use_f32r=None, mode=None, xbufs=None,
                   wbufs=None, pipelined=None):
    import os

    import concourse.bacc as bacc
    import concourse.mybir as mybir
    from concourse.tile import TileContext

    f32 = mybir.dt.float32
    f32r = mybir.dt.float32r
    AFT = mybir.ActivationFunctionType
    Alu = mybir.AluOpType
    if use_f32r is None:
        use_f32r = os.environ.get("BUTTERFLY_FP32R", "0") == "1"
    if reps is None:
        reps = int(os.environ.get("BUTTERFLY_REPS", "1"))
    if mode is None:
        mode = os.environ.get("BUTTERFLY_MODE", "full")  # full|dma|compute
    if xbufs is None:
        xbufs = int(os.environ.get("BUTTERFLY_XBUFS", "3"))
    if wbufs is None:
        wbufs = int(os.environ.get("BUTTERFLY_WBUFS", "4"))
    if pipelined is None:
        pipelined = os.environ.get("BUTTERFLY_PIPE", "1") == "1"
    pybufs = int(os.environ.get("BUTTERFLY_PYBUFS", "2"))
    pobufs = int(os.environ.get("BUTTERFLY_POBUFS", "2"))
    odma = os.environ.get("BUTTERFLY_ODMA", "sp")  # sp | act | pool
    wtile = int(os.environ.get("BUTTERFLY_W", "1024"))
    interpose = os.environ.get("BUTTERFLY_INTERPOSE", "0") == "1"
    xw = int(os.environ.get("BUTTERFLY_XW", "1024"))  # x-load chunk width

    W = 1024                    # megatile width (2 PSUM banks)
    n_wtiles = N_COLS // W      # 8 per row-group

    fmm = f32r if use_f32r else f32

    def mm_cast(ap):
        return ap

    nc = bacc.Bacc("TRN2", target_bir_lowering=False)
    x = nc.dram_tensor("x", [ROWS_PER_CORE, N_COLS], fmm, kind="ExternalInput")
    win = nc.dram_tensor("win", [128, GROUPS_PER_CORE * 128], fmm,
                         kind="ExternalInput")
    wout = nc.dram_tensor("wout", [128, GROUPS_PER_CORE * 128], fmm,
                          kind="ExternalInput")
    biasv = nc.dram_tensor("biasv", [128, GROUPS_PER_CORE], f32,
                           kind="ExternalInput")
    mvec = nc.dram_tensor("mvec", [128, 1], f32, kind="ExternalInput")
    sqbv = nc.dram_tensor("sqbv", [128, 1], f32, kind="ExternalInput")
    yout = nc.dram_tensor("yout", [ROWS_PER_CORE, N_COLS], f32,
                          kind="ExternalOutput")

    with TileContext(nc) as tc:
        with (
            tc.tile_pool(name="consts", bufs=1) as cpool,
            tc.tile_pool(name="xin", bufs=xbufs) as xpool,
            tc.tile_pool(name="work", bufs=wbufs) as wpool,
            tc.tile_pool(name="psum_y", bufs=pybufs, space="PSUM") as pypool,
            tc.tile_pool(name="psum_o", bufs=pobufs, space="PSUM") as popool,
        ):
            win_sb = cpool.tile([128, GROUPS_PER_CORE * 128], fmm)
            wout_sb = cpool.tile([128, GROUPS_PER_CORE * 128], fmm)
            bias_sb = cpool.tile([128, GROUPS_PER_CORE], f32)
            m_sb = cpool.tile([128, 1], f32)
            sqb_sb = cpool.tile([128, 1], f32)
            # group-0 weights first so the first matmul can start early;
            # remaining groups stream in behind the first x tiles.
            g0 = slice(0, 128)
            nc.sync.dma_start(win_sb[:, g0], win[:, g0])
            nc.sync.dma_start(wout_sb[:, g0], wout[:, g0])
            nc.sync.dma_start(bias_sb[:], biasv[:])
            nc.sync.dma_start(m_sb[:], mvec[:])
            nc.sync.dma_start(sqb_sb[:], sqbv[:])
            grest = slice(128, GROUPS_PER_CORE * 128)
            nc.scalar.dma_start(win_sb[:, grest], win[:, grest])
            nc.scalar.dma_start(wout_sb[:, grest], wout[:, grest])

            import contextlib

            stag = os.environ.get("BUTTERFLY_STAG", "0") == "1"
            loop_cm = (tc.For_i(0, reps, 1, staggered_reset=stag)
                       if reps > 1 else contextlib.nullcontext())
            with loop_cm:
                if mode == "full":
                    _emit_body(nc, tc, mybir, x, yout, win_sb, wout_sb,
                               bias_sb, m_sb, sqb_sb, xpool, wpool, pypool,
                               popool, fmm, pipelined=pipelined, odma=odma,
                               W=wtile, interpose=interpose, XW=xw)
                elif mode == "tiny":
                    xt = xpool.tile([128, 1024], fmm, name="xt")
                    nc.sync.dma_start(xt[:], x[0:128, 0:1024])
                    nc.sync.dma_start(yout[0:128, 0:1024], xt[:])
                elif mode == "dma":
                    W = 1024
                    for g in range(GROUPS_PER_CORE):
                        rows = slice(g * 128, (g + 1) * 128)
                        for j in range(N_COLS // W):
                            cols = slice(j * W, (j + 1) * W)
                            xt = xpool.tile([128, W], fmm, name="xt")
                            nc.sync.dma_start(xt[:], x[rows, cols])
                            nc.sync.dma_start(yout[rows, cols], xt[:])
                elif mode == "dmaflat":
                    xf = x[:].flatten().rearrange(
                        "(n p c) -> n p c", p=128, c=1024)
                    yf = yout[:].flatten().rearrange(
                        "(n p c) -> n p c", p=128, c=1024)
                    for i in range(xf.shape[0]):
                        xt = xpool.tile([128, 1024], fmm, name="xt")
                        nc.sync.dma_start(xt[:], xf[i])
                        nc.sync.dma_start(yf[i], xt[:])
                elif mode == "dmabig":
                    for g in range(GROUPS_PER_CORE):
                        rows = slice(g * 128, (g + 1) * 128)
                        xb = xpool.tile([128, N_COLS], fmm, name="xb",
                                        bufs=2)
                        nc.sync.dma_start(xb[:], x[rows, :])
                        nc.sync.dma_start(yout[rows, :], xb[:])
                elif mode == "compute":
                    xc = cpool.tile([128, 1024], fmm, name="xc")
                    nc.vector.memset(xc[:], 1.0)
                    _emit_body(nc, tc, mybir, None, None, win_sb, wout_sb,
                               bias_sb, m_sb, sqb_sb, None, wpool, pypool,
                               popool, fmm, xc=xc)
                elif mode == "inpe":
                    # in-DMA + stage-1 MMs + PSUM evac only
                    for g in range(GROUPS_PER_CORE):
                        lhs1 = win_sb[:, g * 128:(g + 1) * 128]
                        for j in range(N_COLS // 1024):
                            xt = xpool.tile([128, 1024], fmm, name="xt")
                            nc.sync.dma_start(
                                xt[:], x[g * 128:(g + 1) * 128,
                                         j * 1024:(j + 1) * 1024])
                            py = pypool.tile([128, 1024], f32, name="py")
                            for h in range(2):
                                cs = slice(h * 512, (h + 1) * 512)
                                nc.tensor.matmul(py[:, cs], lhs1, xt[:, cs],
                                                 start=True, stop=True)
                            yt = wpool.tile([128, 1024], f32, name="yt")
                            nc.scalar.activation(
                                yt[:], py[:],
                                mybir.ActivationFunctionType.Identity,
                                bias=bias_sb[:, g:g + 1], scale=1.0)
                elif mode == "indep":
                    # compute from a memset tile + unconsumed in-DMAs:
                    # isolates DMA-write/engine contention from deps
                    xc = cpool.tile([128, 1024], fmm, name="xc")
                    nc.vector.memset(xc[:], 1.0)
                    for g in range(GROUPS_PER_CORE):
                        for j in range(N_COLS // 1024):
                            dummy = xpool.tile([128, 1024], fmm,
                                               name="dummy")
                            nc.sync.dma_start(
                                dummy[:],
                                x[g * 128:(g + 1) * 128,
                                  j * 1024:(j + 1) * 1024])
                    _emit_body(nc, tc, mybir, None, None, win_sb, wout_sb,
                               bias_sb, m_sb, sqb_sb, None, wpool, pypool,
                               popool, fmm, xc=xc, pipelined=pipelined,
                               odma=odma)
                elif mode == "noout":
                    _emit_body(nc, tc, mybir, x, None, win_sb, wout_sb,
                               bias_sb, m_sb, sqb_sb, xpool, wpool, pypool,
                               popool, fmm, pipelined=pipelined, odma=odma,
                               interpose=interpose, XW=xw)
                elif mode == "noin":
                    xc = cpool.tile([128, 1024], fmm, name="xc")
                    nc.vector.memset(xc[:], 1.0)
                    _emit_body(nc, tc, mybir, None, yout, win_sb, wout_sb,
                               bias_sb, m_sb, sqb_sb, None, wpool, pypool,
                               popool, fmm, xc=xc, pipelined=pipelined,
                               odma=odma)

    nc.compile()
    return nc


def _emit_body(nc, tc, mybir, x, yout, win_sb, wout_sb, bias_sb, m_sb, sqb_sb,
               xpool, wpool, pypool, popool, fmm, xc=None, pipelined=True,
               odma="sp", W=1024, interpose=False, XW=None):
    f32 = mybir.dt.float32
    AFT = mybir.ActivationFunctionType
    Alu = mybir.AluOpType
    n_wtiles = N_COLS // W
    if XW is None or XW < W:
        XW = W
    tiles_per_xw = XW // W

    # Software-pipelined: stage 2 of megatile k-1 is emitted after the
    # elementwise chain of megatile k, so PE never waits on z.
    tiles = [(g, j) for g in range(GROUPS_PER_CORE) for j in range(n_wtiles)]
    pending = None  # (g, j, zt)
    it = 0
    out_eng = {"sp": nc.sync, "act": nc.scalar, "pool": nc.gpsimd}[odma]

    def stage2(g, j, zt, it):
        lhs2 = wout_sb[:, g * 128:(g + 1) * 128]
        po = popool.tile([128, W], f32, name="po")
        for h in range(W // FREE):
            cs = slice(h * FREE, (h + 1) * FREE)
            nc.tensor.matmul(po[:, cs], lhs2, zt[:, cs],
                             start=True, stop=True)
        ot = wpool.tile([128, W], f32, name="ot")
        if it % 2 == 0:
            nc.vector.tensor_copy(ot[:], po[:])
        else:
            nc.scalar.copy(ot[:], po[:])
        if yout is not None:
            out_eng.dma_start(
                yout[g * 128:(g + 1) * 128, j * W:(j + 1) * W], ot[:])

    for (g, j) in tiles:
        rows = slice(g * 128, (g + 1) * 128)
        cols = slice(j * W, (j + 1) * W)
        lhs1 = win_sb[:, g * 128:(g + 1) * 128]
        bias_g = bias_sb[:, g:g + 1]
        if xc is not None:
            xt = xc
        else:
            if j % tiles_per_xw == 0:
                xt_wide = xpool.tile([128, XW], fmm, name="xt")
                nc.sync.dma_start(
                    xt_wide[:], x[rows, j * W:j * W + XW])
                if interpose:
                    xt2 = xpool.tile([128, XW], fmm, name="xt2")
                    nc.gpsimd.tensor_copy(xt2[:], xt_wide[:])
                    xt_wide = xt2
            off = (j % tiles_per_xw) * W
            xt = xt_wide[:, off:off + W]

        # stage 1: y' = diag(d).Min @ x + d*b   (one MM per PSUM bank)
        py = pypool.tile([128, W], f32, name="py")
        for h in range(W // FREE):
            cs = slice(h * FREE, (h + 1) * FREE)
            nc.tensor.matmul(py[:, cs], lhs1, xt[:, cs],
                             start=True, stop=True)
        yt = wpool.tile([128, W], f32, name="yt")
        nc.scalar.activation(yt[:], py[:], AFT.Identity,
                             bias=bias_g, scale=1.0)

        # t2 = y'^2 ; s = sqrt(m*t2 + (m*0.05)^2) ; z = y' + s
        tt = wpool.tile([128, W], f32, name="tt")
        nc.vector.tensor_tensor(tt[:], yt[:], yt[:], Alu.mult)
        st = wpool.tile([128, W], f32, name="st")
        nc.scalar.activation(st[:], tt[:], AFT.Sqrt,
                             bias=sqb_sb[:, 0:1],
                             scale=m_sb[:, 0:1])
        zt = wpool.tile([128, W], fmm, name="zt")
        nc.vector.tensor_tensor(zt[:], yt[:], st[:], Alu.add)

        if not pipelined:
            stage2(g, j, zt, it)
            it += 1
        else:
            if pending is not None:
                stage2(*pending, it)
                it += 1
            pending = (g, j, zt)

    if pending is not None:
        stage2(*pending, it)


def _get_program():
    if "nc" not in _PROGRAM_CACHE:
        _PROGRAM_CACHE["nc"] = _build_program()
    return _PROGRAM_CACHE["nc"]


def _prepare_in_maps(inputs):
    data = np.asarray(inputs["data"])
    x_full = np.ascontiguousarray(
        np.asarray(data, np.float32)[np.asarray(inputs["indices_in"])]
    )
    weights = _host_weights(inputs["angles"], inputs["biases"])
    in_maps = []
    for c in range(N_CORES):
        im = dict(weights[c])
        im["x"] = np.ascontiguousarray(
            x_full[c * ROWS_PER_CORE:(c + 1) * ROWS_PER_CORE]
        )
        in_maps.append(im)
    return in_maps


def kernel(data, angles, biases, indices_in, idx_out, _return_results=False):
    from concourse import bass_utils

    data = np.asarray(data)
    in_maps = _prepare_in_maps(
        {"data": data, "angles": angles, "biases": biases,
         "indices_in": indices_in}
    )

    nc = _get_program()
    res = bass_utils.run_bass_kernel_spmd(nc, in_maps,
                                          core_ids=list(range(N_CORES)))
    y = np.concatenate([res.results[c]["yout"] for c in range(N_CORES)], axis=0)
    out = np.array(data, copy=True)
    out[np.asarray(idx_out)] = y
    if _return_results:
        return out, res
    return out



# revision 7
# speedup vs baseline: 9.1916x; 9.1916x over previous
"""Trainium2 Bass kernel for the ButterflyModule problem.

Semantics (N=4096 rows, B=8192 cols):
  x = data[indices_in]
  4 Givens-rotation butterfly layers (strides 1,2,4,8 within 16-row blocks)
  bias + smooth-ReLU on rows with (row%16)<8
  4 more butterfly layers (strides 1,2,4,8)
  out = data with rows idx_out replaced by the result

Device strategy (fp16 I/O, exact math):

The 4 input layers compose to a 16x16 matrix Min per block, the 4 output
layers to Mout.  With A = act rows (o<8 within each block) and
xa = (Min x)_A + b, the activated rows satisfy

  z_A - y_A = -0.5 y_A + 0.5 b + 0.5 w',   w' = sqrt(xa^2 + c^2)

so with C2 = Mout Min - 0.5 MoutA MinA (all block-diagonal composes,
host-side in fp64):

  out = C2 x + (0.5 MoutA) w' + MoutA (0.5 b)

Per 1024-col tile that is: 3 matmuls (yA = MinA x; po = C2 x; po += wmo w'),
an ACT Square (xa^2, bias folded) + ACT Sqrt, and a PSUM->SBUF evac with the
d = MoutA(0.5 b) constant folded in, split column-wise between ACT and DVE.

Act-row tiles are HALF-height [64, W], so two consecutive tiles are packed
into one [128, W] tile (partitions 0:64 = even tile, 64:128 = odd tile,
using PE tile_position quadrants; wmo is duplicated on partitions 64:128 for
the odd-tile accumulate).  This halves the ACT cost of the Square/Sqrt pair.

All I/O is fp16 (host casts), halving HBM traffic: 8 MiB in + 8 MiB out per
core ~= 47 us at 358 GB/s, with PE ~41 us, ACT ~31 us, DVE ~30 us hidden
under it.  Input loads go on the SP HWDGE ring, output stores on the ACT
HWDGE ring so the two directions never queue behind each other.

Rows are sharded across the 8 cores (512 rows each); rotations never cross
16-row block boundaries so there is no cross-core communication.
"""

import sys

if "/opt/trn_rl_repo" not in sys.path:
    sys.path.insert(0, "/opt/trn_rl_repo")

import numpy as np

N_ROWS = 4096
N_COLS = 8192
COL_BLOCK = 16
NUM_ACT = 8
CURVATURE = 0.1
N_CORES = 8
ROWS_PER_CORE = N_ROWS // N_CORES          # 512
GROUPS_PER_CORE = ROWS_PER_CORE // 128     # 4
BLOCKS_PER_GROUP = 128 // COL_BLOCK        # 8
W = 1024                                   # compute tile width
FREE = 512                                 # PSUM-bank-limited matmul free dim
N_WTILES = N_COLS // W                     # 8 per group

_PROGRAM_CACHE = {}


def _butterfly_mats(angles64):
    """Compose butterfly layers into per-block 16x16 matrices.

    angles64: [8, 2048] float64.  Returns (Min, Mout) each [256, 16, 16],
    where layer l uses stride 1<<(l%4) and block b uses angles[l, 8b:8b+8]
    ordered by the low row index within the block.
    """
    nb = N_ROWS // COL_BLOCK

    def accum(l0, l1):
        G = np.broadcast_to(np.eye(COL_BLOCK), (nb, COL_BLOCK, COL_BLOCK)).copy()
        for l in range(l0, l1):
            stride = 1 << (l % 4)
            offs = [o for o in range(COL_BLOCK) if (o & stride) == 0]
            a = angles64[l].reshape(nb, NUM_ACT)
            c = np.cos(a)
            s = np.sin(a)
            for k, o in enumerate(offs):
                gl = G[:, o, :].copy()
                gh = G[:, o + stride, :].copy()
                G[:, o, :] = c[:, k, None] * gl + s[:, k, None] * gh
                G[:, o + stride, :] = -s[:, k, None] * gl + c[:, k, None] * gh
        return G

    return accum(0, 4), accum(4, 8)


def _host_weights(angles, biases, act_mode="exact"):
    """Per-core weight tensors for the device kernel.

    Per 128-row group g (8 blocks), with A = act rows (o<8, ordered
    p=8i+o for block i, offset o):
      wc2:  lhsT of block-diag(C2)                        [128, G*128] fp16
      wma:  lhsT of MinA  (rows A of block-diag Min)      [128, G*64]  fp16
      wmo:  lhsT of s*MoutA, duplicated on both halves    [128, G*128] fp16
      bact: ACT bias vector (b, duplicated both halves)   [128, G] fp32
      dvec: per-partition additive constant for the evac  [128, G] fp32
    exact mode: C2 = Mout Min - 0.5 MoutA MinA, s = 0.5, dvec = MoutA(0.5 b),
                device computes w' = sqrt((yA+b)^2+c^2) via Square+Sqrt.
    relu mode:  C2 = Mout Min, s = 1, dvec = MoutA b, bact = -b,
                device computes w' = relu(-(yA+b)) in one ACT op (approx).
    """
    ang64 = np.asarray(angles, np.float64)
    b64 = np.asarray(biases, np.float64)
    Min, Mout = _butterfly_mats(ang64)

    per_core = []
    for c in range(N_CORES):
        wc2 = np.zeros((128, GROUPS_PER_CORE * 128))
        wma = np.zeros((128, GROUPS_PER_CORE * 64))
        wmo = np.zeros((128, GROUPS_PER_CORE * 128))
        bact = np.zeros((128, GROUPS_PER_CORE))
        dvec = np.zeros((128, GROUPS_PER_CORE))
        for g in range(GROUPS_PER_CORE):
            G = c * GROUPS_PER_CORE + g
            minbd = np.zeros((128, 128))
            moutbd = np.zeros((128, 128))
            for i in range(BLOCKS_PER_GROUP):
                B = G * BLOCKS_PER_GROUP + i
                sl = slice(i * 16, (i + 1) * 16)
                minbd[sl, sl] = Min[B]
                moutbd[sl, sl] = Mout[B]
            # act rows of the group: partition p=8i+o <-> row 16i+o
            arows = np.array([16 * i + o for i in range(8) for o in range(8)])
            mina = minbd[arows, :]               # [64, 128]
            mouta = moutbd[:, arows]             # [128, 64]
            bvec = np.array([
                b64[(G * 8 + i) * 8 + o] for i in range(8) for o in range(8)
            ])
            if act_mode == "exact":
                c2 = moutbd @ minbd - 0.5 * (mouta @ mina)
                wmo_g = (0.5 * mouta).T
                bact_g = bvec
                dvec_g = mouta @ (0.5 * bvec)
            else:  # relu approximation
                c2 = moutbd @ minbd
                wmo_g = mouta.T
                bact_g = -bvec
                dvec_g = mouta @ bvec
            wc2[:, g * 128:(g + 1) * 128] = c2.T
            wma[:, g * 64:(g + 1) * 64] = mina.T
            wmo[0:64, g * 128:(g + 1) * 128] = wmo_g
            wmo[64:128, g * 128:(g + 1) * 128] = wmo_g
            bact[0:64, g] = bact_g
            bact[64:128, g] = bact_g
            dvec[:, g] = dvec_g
        per_core.append({
            "wc2": np.ascontiguousarray(wc2, np.float16),
            "wma": np.ascontiguousarray(wma, np.float16),
            "wmo": np.ascontiguousarray(wmo, np.float16),
            "bact": np.ascontiguousarray(bact, np.float32),
            "dvec": np.ascontiguousarray(dvec, np.float32),
        })
    return per_core


def _build_program(reps=None, mode=None, act_mode=None, xw=None, ow=None,
                   fs=None, odma=None):
    import os
    import contextlib

    import concourse.bacc as bacc
    import concourse.mybir as mybir
    from concourse.tile import TileContext

    f32 = mybir.dt.float32
    f16 = mybir.dt.float16
    if reps is None:
        reps = int(os.environ.get("BUTTERFLY_REPS", "1"))
    if mode is None:
        mode = os.environ.get("BUTTERFLY_MODE", "full")
    if act_mode is None:
        act_mode = os.environ.get("BUTTERFLY_ACT", "exact")
    if xw is None:
        xw = int(os.environ.get("BUTTERFLY_XW", "4096"))
    if ow is None:
        ow = int(os.environ.get("BUTTERFLY_OW", "2048"))
    if fs is None:
        fs = int(os.environ.get("BUTTERFLY_FS", "128"))
    if odma is None:
        odma = os.environ.get("BUTTERFLY_ODMA", "act")

    nc = bacc.Bacc("TRN2", target_bir_lowering=False)
    x = nc.dram_tensor("x", [ROWS_PER_CORE, N_COLS], f16, kind="ExternalInput")
    wc2 = nc.dram_tensor("wc2", [128, GROUPS_PER_CORE * 128], f16,
                         kind="ExternalInput")
    wma = nc.dram_tensor("wma", [128, GROUPS_PER_CORE * 64], f16,
                         kind="ExternalInput")
    wmo = nc.dram_tensor("wmo", [128, GROUPS_PER_CORE * 128], f16,
                         kind="ExternalInput")
    bact = nc.dram_tensor("bact", [128, GROUPS_PER_CORE], f32,
                          kind="ExternalInput")
    dvec = nc.dram_tensor("dvec", [128, GROUPS_PER_CORE], f32,
                          kind="ExternalInput")
    yout = nc.dram_tensor("yout", [ROWS_PER_CORE, N_COLS], f16,
                          kind="ExternalOutput")

    with TileContext(nc) as tc:
        with (
            tc.tile_pool(name="consts", bufs=1) as cpool,
            tc.tile_pool(name="xin", bufs=3) as xpool,
            tc.tile_pool(name="wbuf", bufs=3) as wpool,
            tc.tile_pool(name="obuf", bufs=2) as opool,
            tc.tile_pool(name="psum_y", bufs=2, space="PSUM") as pypool,
            tc.tile_pool(name="psum_o", bufs=2, space="PSUM") as popool,
        ):
            wc2_sb = cpool.tile([128, GROUPS_PER_CORE * 128], f16)
            wma_sb = cpool.tile([128, GROUPS_PER_CORE * 64], f16)
            wmo_sb = cpool.tile([128, GROUPS_PER_CORE * 128], f16)
            bact_sb = cpool.tile([128, GROUPS_PER_CORE], f32)
            dvec_sb = cpool.tile([128, GROUPS_PER_CORE], f32)
            nc.sync.dma_start(wma_sb[:], wma[:])
            nc.sync.dma_start(wc2_sb[:], wc2[:])
            nc.sync.dma_start(wmo_sb[:], wmo[:])
            nc.sync.dma_start(bact_sb[:], bact[:])
            nc.sync.dma_start(dvec_sb[:], dvec[:])
            csq_sb = cpool.tile([128, 1], f32)
            nc.vector.memset(csq_sb[:], float(CURVATURE) ** 2)

            loop_cm = (tc.For_i(0, reps, 1) if reps > 1
                       else contextlib.nullcontext())
            with loop_cm:
                if mode == "full":
                    _emit_body(nc, tc, mybir, x, yout, wc2_sb, wma_sb, wmo_sb,
                               bact_sb, dvec_sb, csq_sb, xpool, wpool, opool,
                               pypool, popool, act_mode=act_mode, xw=xw,
                               ow=ow, fs=fs, odma=odma)
                elif mode == "dma":
                    for g in range(GROUPS_PER_CORE):
                        rows = slice(g * 128, (g + 1) * 128)
                        for j in range(N_COLS // xw):
                            cols = slice(j * xw, (j + 1) * xw)
                            xt = xpool.tile([128, xw], f16, name="xt")
                            nc.sync.dma_start(xt[:], x[rows, cols])
                            nc.scalar.dma_start(yout[rows, cols], xt[:])
                elif mode == "noout":
                    _emit_body(nc, tc, mybir, x, None, wc2_sb, wma_sb,
                               wmo_sb, bact_sb, dvec_sb, csq_sb, xpool, wpool,
                               opool, pypool, popool, act_mode=act_mode,
                               xw=xw, ow=ow, fs=fs, odma=odma)
                elif mode == "noin":
                    _emit_body(nc, tc, mybir, None, yout, wc2_sb, wma_sb,
                               wmo_sb, bact_sb, dvec_sb, csq_sb, xpool, wpool,
                               opool, pypool, popool, act_mode=act_mode,
                               xw=xw, ow=ow, fs=fs, odma=odma)
                elif mode == "compute":
                    _emit_body(nc, tc, mybir, None, None, wc2_sb, wma_sb,
                               wmo_sb, bact_sb, dvec_sb, csq_sb, xpool, wpool,
                               opool, pypool, popool, act_mode=act_mode,
                               xw=xw, ow=ow, fs=fs, odma=odma)

    nc.compile()
    return nc


def _emit_body(nc, tc, mybir, x, yout, wc2_sb, wma_sb, wmo_sb, bact_sb,
               dvec_sb, csq_sb, xpool, wpool, opool, pypool, popool, act_mode,
               xw, ow, fs, odma):
    f32 = mybir.dt.float32
    f16 = mybir.dt.float16
    AFT = mybir.ActivationFunctionType
    Alu = mybir.AluOpType
    tiles_per_xw = xw // W
    tiles_per_ow = ow // W
    out_eng = {"sp": nc.sync, "act": nc.scalar, "pool": nc.gpsimd}[odma]

    if x is None:
        xc, _ = tc.tile([128, 2 * W], f16, name="xc")
        nc.vector.memset(xc[:], 0.25)

    state = {"xt": None, "ot": None, "pending": None}

    def evac_one(g, j, po):
        # po -> ot (fp16) with the dvec constant added; cols split ACT/DVE
        if j % tiles_per_ow == 0:
            state["ot"] = opool.tile([128, ow], f16, name="ot")
        ot = state["ot"]
        off = (j % tiles_per_ow) * W
        dv = dvec_sb[:, g:g + 1]
        if fs > 0:
            nc.scalar.activation(ot[:, off:off + fs], po[:, 0:fs],
                                 AFT.Identity, bias=dv, scale=1.0)
        if fs < W:
            nc.vector.tensor_scalar(ot[:, off + fs:off + W], po[:, fs:W],
                                    dv, None, Alu.add)
        if yout is not None and (j + 1) % tiles_per_ow == 0:
            j0 = (j // tiles_per_ow) * tiles_per_ow
            out_eng.dma_start(
                yout[g * 128:(g + 1) * 128, j0 * W:j0 * W + ow], ot[:])

    def finish(g, j0, xta, xtb, wt2):
        # deferred stage: po = C2 x (+= wmo w'), evac, store; two tiles
        lhs_c2 = wc2_sb[:, g * 128:(g + 1) * 128]
        for half, (xt, j) in enumerate(((xta, j0), (xtb, j0 + 1))):
            po = popool.tile([128, W], f32, name="po")
            lhs_mo = wmo_sb[64 * half:64 * (half + 1),
                            g * 128:(g + 1) * 128]
            wts = wt2[64 * half:64 * (half + 1), :]
            for h in range(W // FREE):
                cs = slice(h * FREE, (h + 1) * FREE)
                nc.tensor.matmul(po[:, cs], lhs_c2, xt[:, cs],
                                 start=True, stop=False)
                nc.tensor.matmul(po[:, cs], lhs_mo, wts[:, cs],
                                 start=False, stop=True)
            evac_one(g, j, po)

    for g in range(GROUPS_PER_CORE):
        rows = slice(g * 128, (g + 1) * 128)
        lhs_ma = wma_sb[:, g * 64:(g + 1) * 64]
        bn = bact_sb[:, g:g + 1]
        for jp in range(N_WTILES // 2):
            ja, jb = 2 * jp, 2 * jp + 1
            if x is not None:
                xts = []
                for j in (ja, jb):
                    if j % tiles_per_xw == 0:
                        state["xt"] = xpool.tile([128, xw], f16, name="xt")
                        nc.sync.dma_start(
                            state["xt"][:], x[rows, j * W:j * W + xw])
                    off = (j % tiles_per_xw) * W
                    xts.append(state["xt"][:, off:off + W])
                xta, xtb = xts
            else:
                xta, xtb = xc[:, 0:W], xc[:, W:2 * W]

            # yA for both tiles packed into one [128, W] PSUM tile
            py2 = pypool.tile([128, W], f32, name="py2")
            for h in range(W // FREE):
                cs = slice(h * FREE, (h + 1) * FREE)
                nc.tensor.matmul(py2[0:64, cs], lhs_ma, xta[:, cs],
                                 start=True, stop=True)
            for h in range(W // FREE):
                cs = slice(h * FREE, (h + 1) * FREE)
                nc.tensor.matmul(py2[64:128, cs], lhs_ma, xtb[:, cs],
                                 start=True, stop=True)

            wt2 = wpool.tile([128, W], f16, name="wt2")
            if act_mode == "exact":
                # w' = sqrt((yA + b)^2 + c^2)
                sq = wpool.tile([128, W], f32, name="sq")
                nc.scalar.activation(sq[:], py2[:], AFT.Square,
                                     bias=bn, scale=1.0)
                nc.scalar.activation(wt2[:], sq[:], AFT.Sqrt,
                                     bias=csq_sb[:, 0:1], scale=1.0)
            else:
                # w' = relu(-(yA + b))   (bact = -b on the host side)
                nc.scalar.activation(wt2[:], py2[:], AFT.Relu,
                                     bias=bn, scale=-1.0)

            if state["pending"] is not None:
                finish(*state["pending"])
            state["pending"] = (g, ja, xta, xtb, wt2)

    if state["pending"] is not None:
        finish(*state["pending"])


def _get_program():
    if "nc" not in _PROGRAM_CACHE:
        _PROGRAM_CACHE["nc"] = _build_program()
    return _PROGRAM_CACHE["nc"]


def _prepare_in_maps(inputs):
    import os

    act_mode = os.environ.get("BUTTERFLY_ACT", "exact")
    data = np.asarray(inputs["data"])
    x_full = np.asarray(data, np.float32)[np.asarray(inputs["indices_in"])]
    x16 = np.ascontiguousarray(x_full.astype(np.float16))
    weights = _host_weights(inputs["angles"], inputs["biases"],
                            act_mode=act_mode)
    in_maps = []
    for c in range(N_CORES):
        im = dict(weights[c])
        im["x"] = np.ascontiguousarray(
            x16[c * ROWS_PER_CORE:(c + 1) * ROWS_PER_CORE]
        )
        in_maps.append(im)
    return in_maps


def kernel(data, angles, biases, indices_in, idx_out, _return_results=False):
    from concourse import bass_utils

    data = np.asarray(data)
    in_maps = _prepare_in_maps(
        {"data": data, "angles": angles, "biases": biases,
         "indices_in": indices_in}
    )

    nc = _get_program()
    res = bass_utils.run_bass_kernel_spmd(nc, in_maps,
                                          core_ids=list(range(N_CORES)))
    y = np.concatenate(
        [res.results[c]["yout"] for c in range(N_CORES)], axis=0
    ).astype(np.float32)
    out = np.array(data, copy=True)
    out[np.asarray(idx_out)] = y
    if _return_results:
        return out, res
    return out
